# revision 81
# baseline (speedup 1.0000x reference)
"""Adaptive-style-attention (AdaAttN-like) Trainium2 kernel, 8 NeuronCores.

Math (per batch b, with N = M = 64*64 = 4096 pixels, C = Ck = 256):
  Fq = Wf @ content_key[b] + bf          # [C, N]   (q^T)
  G  = Wg @ style_key[b]   + bg          # [C, M]   (k)
  Hv = Wh @ style[b]       + bh          # [C, M];  V = Hv^T  [M, C]
  S  = softmax_m(q @ k)                  # [N, M]
  mean = S @ V ; e2 = S @ V^2            # [N, C]
  std  = sqrt(relu(e2 - mean^2))
  out  = std * mvn(content[b]) + mean    # [C, N] layout

Sharding: 8 cores = batch(4) x query-halves(2). Each core computes its
2048 query rows against the full 4096 style pixels of its batch.

Everything is computed transposed ([c, n] / [m, n] layouts) so no
on-chip transposes are needed:
  logits^T tile [m=128, n=512] = G_chunk.T @ Fq_chunk   (K = c)
  P^T = exp(logits^T - SHIFT)  (global shift; logits ~ N(0, 256))
  mean^T [c, n] += (V[m, c])-as-lhsT @ P^T  (K = m), PSUM-accumulated

Dtypes (all verified on-silicon against the 2e-2 budget; measured
rel err ~4.3e-3):
 - Stationary matmul operands are FP16 (G, V, V^2, conv weights and
   conv inputs): fp16's 2-byte LDWEIGHTS loads in ~90ns vs ~190ns for
   the 4-byte f32/f32r path, and that LDW issue rate - not the matmul
   streaming - paced the old all-f32r kernel.  fp16's 5e-4 rounding is
   ~8x finer than bf16, which keeps the e2 - mean^2 cancellation sane.
 - P = exp(logits - 48) is BF16 (needs f32-class exponent range: fp16
   over/underflows under a global shift).  The SAME bf16 P feeds
   rowsum, mean and e2, so its rounding largely cancels through the
   softmax normalization; V/V^2 stay fp16 so the V-vs-V^2 rounding
   stays consistent.  fp16 lhsT x bf16 rhs is a legal PE mix (only
   f32/f32r must match dtypes).
 - Rowsum accumulation stays FP32 (a rowsum scale error eps shows up
   in var as eps*(mean^2 - var), catastrphic where std is small).

Scheduling (the Tile scheduler is an out-of-order, priority-tie-break
list scheduler driven by a sim - emission order sets priority only):
 - PE floor is ~191us/core: 883 matmuls, all streaming-bound at one
   16-bit column/cycle (N=512 -> 216ns back-to-back, measured).
 - nm=1..3 run as ONE flat software-pipelined loop: logits+exp emitted
   LA=3 tiles ahead of their PV consumer (lps pool = 4 PSUM banks,
   shared with convs/warm/rowsum tiles; accumulators take the other 4),
   so PV never waits on the exp handoff and the pipeline crosses nm
   boundaries without a bubble.
 - e2 matmuls are skewed ESKEW=3 tiles behind mean so only the two
   mean-bank DVE drains are boundary-critical; this keeps the PE idle
   gap below the ~3.4us HAM window (no mid-kernel clock-down).
 - The rowsum reduce + reciprocal + var/std epilogues are hooked ~8-19
   tiles INTO the next n-macro so the ACT sqrt (with its 2x ~1.3us
   table reloads) never parks in the boundary idle slot where it would
   head-of-line-block the next n-macro's exps.
 - Engine balance: exp on ACT; conv bias drains on DVE; softmax
   rowsum partial sums split DVE (even mt) / GpSimd (odd mt); V^2
   squares and half the tail on GpSimd; mvn stats via DVE bn_stats/
   bn_aggr; epilogue squares/relu on DVE (TENSOR_SCALAR MAX with a
   strided AP measures ~8x slow - use tensor_tensor max vs zeros).
 - Weights DMA split 3 ways, final output DMAs quarter-split across
   both issue queues; content chunks DMA straight into ct_sh.
 - Chunk staging (DMA + convs) is emitted one chunk AHEAD of the
   attention tiles consuming the previous chunk, and the own-half
   content bn_stats are deferred into quiet nm=1 slots: both keep the
   j-loop's DVE/PE off the logits-drain critical path.
"""
import os
import numpy as np

import concourse.bass as bass
import concourse.mybir as mybir
import concourse.tile as tile
from concourse import bacc
from concourse.bass_utils import run_bass_kernel_spmd

B, C, HW = 4, 256, 64 * 64          # N = M = HW
NSH = HW // 2                        # queries per core = 2048
SHIFT = 48.0
EPS = 1e-5
F32 = mybir.dt.float32
F32R = mybir.dt.float32r
BF16 = mybir.dt.bfloat16
F16 = mybir.dt.float16
AF = mybir.ActivationFunctionType

NM = 4          # n macro tiles of 512 within the 2048-query shard
MT = 32         # m tiles of 128 within 4096 style pixels
NW = 512
NCH = 8         # style chunks of 512 m

_last_result = {}


def _build_nc() -> bass.Bass:
    nc = bacc.Bacc("TRN2", target_bir_lowering=False)
    # chunk-contiguous layouts: [chunk][p][k][col] so each chunk DMA is one
    # 4KB-per-partition contiguous block.
    ck = nc.dram_tensor("ck", [NM, 128, 2, NW], F16, kind="ExternalInput")
    ct = nc.dram_tensor("ct", [NCH, 128, 2, NW], F32, kind="ExternalInput")     # rotated content (own half first)
    sk = nc.dram_tensor("sk", [NCH, 128, 2, NW], F16, kind="ExternalInput")
    st = nc.dram_tensor("st", [NCH, 128, 2, NW], F16, kind="ExternalInput")
    wall = nc.dram_tensor("wall", [128, 3, 2, C], F16, kind="ExternalInput")    # W{f,g,h}^T
    ball = nc.dram_tensor("ball", [128, 4 + C], F32, kind="ExternalInput")      # bf[2] bg[2] bh[256]
    out = nc.dram_tensor("out", [NM, 2, 128, NW], F32, kind="ExternalOutput")   # [nm, c-chunk, p, n]

    with tile.TileContext(nc, pool_alloc_mode="queue") as tc:
        _emit(nc, tc, ck, ct, sk, st, wall, ball, out)
    nc.compile()
    return nc


def _emit(nc, tc, ck, ct, sk, st, wall, ball, out):
    from contextlib import ExitStack

    with ExitStack() as ctx:
        # ---------------- pools ----------------
        consts = ctx.enter_context(tc.tile_pool(name="consts", bufs=1))
        fq_p = ctx.enter_context(tc.tile_pool(name="fq", bufs=1))
        g_p = ctx.enter_context(tc.tile_pool(name="g", bufs=1))
        v_p = ctx.enter_context(tc.tile_pool(name="v", bufs=1))
        v2_p = ctx.enter_context(tc.tile_pool(name="v2", bufs=1))
        nrm_p = ctx.enter_context(tc.tile_pool(name="nrm", bufs=1))
        ct_pool = ctx.enter_context(tc.tile_pool(name="ctsh", bufs=1))
        wcon = ctx.enter_context(tc.tile_pool(name="wcon", bufs=1))
        prt = ctx.enter_context(tc.tile_pool(name="prt", bufs=1))
        sml = ctx.enter_context(tc.tile_pool(name="sml", bufs=8))
        chk = ctx.enter_context(tc.tile_pool(name="chk", bufs=8))
        ctchk = ctx.enter_context(tc.tile_pool(name="ctchk", bufs=1))
        pt_pool = ctx.enter_context(tc.tile_pool(name="pt", bufs=8))
        accp = ctx.enter_context(tc.tile_pool(name="accp", bufs=4))
        wrk = ctx.enter_context(tc.tile_pool(name="wrk", bufs=4))
        rsml = ctx.enter_context(tc.tile_pool(name="rsml", bufs=1))
        outp = ctx.enter_context(tc.tile_pool(name="outp", bufs=2))
        lps = ctx.enter_context(tc.tile_pool(name="lpsum", bufs=4, space="PSUM"))
        aps = ctx.enter_context(tc.tile_pool(name="apsum", bufs=4, space="PSUM"))

        # ---------------- persistent tiles ----------------
        fq_s = fq_p.tile([128, 2, NSH], F16)      # [p, c-chunk, n]
        g_s = g_p.tile([128, 2, HW], F16)         # [p, c-chunk, m]
        v_s = v_p.tile([128, MT, C], F16)         # [p(m), m-tile, c]
        v2_s = v2_p.tile([128, MT, C], F16)
        nrm_s = nrm_p.tile([128, 2, NSH], BF16)   # normalized content [p, c-chunk, n]
        ct_sh = ct_pool.tile([128, 2, NSH], F32)  # own-half content (for mvn)

        negshift = consts.tile([128, 1], F32)
        nc.vector.memset(negshift[:], -SHIFT)
        ones_f = consts.tile([128, 128], F32)
        nc.vector.memset(ones_f[:], 1.0)
        ones_r = consts.tile([128, 128], F32R)
        nc.vector.tensor_copy(ones_r[:], ones_f[:])
        ones_h = consts.tile([128, 1], BF16)
        nc.vector.memset(ones_h[:], 1.0)
        zeros_f = consts.tile([128, NW], F32)
        nc.vector.memset(zeros_f[:], 0.0)

        # first two sync-queue DMA issues gate everything: ck chunk 0,
        # then all weights+biases as single descriptors.
        t0 = chk.tile([128, 2, NW], F16, tag="chk", name="ck0")
        nc.sync.dma_start(t0[:], ck[0])
        w_s = wcon.tile([128, 3, 2, C], F16)
        # split per-W so three DMA engines move them in parallel; wf first
        # since fq_conv(0) gates the whole front.
        nc.gpsimd.dma_start(w_s[:, 0], wall[:, 0])
        nc.gpsimd.dma_start(w_s[:, 1], wall[:, 1])
        nc.gpsimd.dma_start(w_s[:, 2], wall[:, 2])
        b_s = wcon.tile([128, 4 + C], F32)
        nc.gpsimd.dma_start(b_s[:], ball[:])
        wf_s, wg_s, wh_s = w_s[:, 0], w_s[:, 1], w_s[:, 2]
        bf_s, bg_s, bh_s = b_s[:, 0:2], b_s[:, 2:4], b_s[:, 4:4 + C]

        # bn_stats records per content chunk: own half cols 0-3, other 4-7
        bnbuf = prt.tile([128, 2, 8, 6], F32)

        # HAM warmup: dummy matmuls (no DMA dependency) ramp the PE clock.
        # ones_f (plain fp32) is ready ~3us before ones_r's copy chain.
        warm_ps = lps.tile([128, NW], F32, tag="log", name="warm_ps")
        for _ in range(8):
            nc.tensor.matmul(warm_ps[:, 0:128], ones_f[:, 0:128], ones_f[:, :],
                             start=True, stop=True)

        # ---------------- conv + stats helpers ----------------
        def fq_conv(j, tr):
            nsl = slice(j * NW, (j + 1) * NW)
            for c2 in range(2):
                ps = lps.tile([128, NW], F32, tag="log")
                for k in range(2):
                    nc.tensor.matmul(
                        ps[:], wf_s[:, k, c2 * 128:(c2 + 1) * 128], tr[:, k, :],
                        start=(k == 0), stop=(k == 1))
                nc.vector.tensor_scalar_add(fq_s[:, c2, nsl], ps[:],
                                            bf_s[:, c2:c2 + 1])

        def g_conv(j, tg):
            msl = slice(j * NW, (j + 1) * NW)
            for c2 in range(2):
                ps = lps.tile([128, NW], F32, tag="log")
                for k in range(2):
                    nc.tensor.matmul(
                        ps[:], wg_s[:, k, c2 * 128:(c2 + 1) * 128], tg[:, k, :],
                        start=(k == 0), stop=(k == 1))
                nc.vector.tensor_scalar_add(g_s[:, c2, msl], ps[:],
                                            bg_s[:, c2:c2 + 1])

        def v_conv(j, tv):
            for sub in range(NM):
                mt = j * NM + sub
                ps = lps.tile([128, NW], F32, tag="log")
                for k in range(2):
                    nc.tensor.matmul(
                        ps[:, 0:C], tv[:, k, sub * 128:(sub + 1) * 128],
                        wh_s[:, k, :], start=(k == 0), stop=(k == 1))
                nc.vector.tensor_add(v_s[:, mt, :], ps[:, 0:C], bh_s[:])
                nc.gpsimd.tensor_mul(v2_s[:, mt, :], v_s[:, mt, :],
                                     v_s[:, mt, :])

        def stats_chunk(src, col):
            for k in range(2):
                nc.vector.bn_stats(bnbuf[:, k, col], src[:, k, :])

        # ---------------- attention ----------------
        mean_ps = [aps.tile([128, NW], F32, tag="acc", name=f"mean_ps{i}")
                   for i in range(2)]
        e2_ps = [aps.tile([128, NW], F32, tag="acc", name=f"e2_ps{i}")
                 for i in range(2)]

        def attn_log(nm, mt):
            nsl = slice(nm * NW, (nm + 1) * NW)
            msl = slice(mt * 128, (mt + 1) * 128)
            ps_l = lps.tile([128, NW], F32, tag="log")
            for k in range(2):
                nc.tensor.matmul(ps_l[:], g_s[:, k, msl], fq_s[:, k, nsl],
                                 start=(k == 0), stop=(k == 1))
            pt = pt_pool.tile([128, NW], BF16, tag="pt")
            nc.scalar.activation(pt[:], ps_l[:], AF.Exp,
                                 bias=negshift[:], scale=1.0)
            return pt

        def attn_pv_mean(mt, pt):
            for c2 in range(2):
                nc.tensor.matmul(mean_ps[c2][:],
                                 v_s[:, mt, c2 * 128:(c2 + 1) * 128],
                                 pt[:], start=(mt == 0), stop=(mt == MT - 1))

        def attn_pv_e2(mt, pt):
            for c2 in range(2):
                nc.tensor.matmul(e2_ps[c2][:],
                                 v2_s[:, mt, c2 * 128:(c2 + 1) * 128],
                                 pt[:], start=(mt == 0), stop=(mt == MT - 1))

        def attn_acc(mt, pt, acc_d, acc_g):
            # rowsum partials: even mt on DVE, odd mt on GpSimd (two chains,
            # merged with pt_last by the PE reduce in epilogue_red)
            if mt == 0:
                nc.vector.tensor_copy(acc_d[:], pt[:])
            elif mt == 1:
                nc.gpsimd.tensor_copy(acc_g[:], pt[:])
            elif mt == MT - 1:
                pass
            elif mt % 2 == 0:
                nc.vector.tensor_add(acc_d[:], acc_d[:], pt[:])
            else:
                nc.gpsimd.tensor_add(acc_g[:], acc_g[:], pt[:])

        def attn_mt(nm, mt, acc_d, acc_g):
            pt = attn_log(nm, mt)
            attn_pv_mean(mt, pt)
            attn_pv_e2(mt, pt)
            attn_acc(mt, pt, acc_d, acc_g)
            return pt

        def epilogue_drain(nm):
            # eager DVE drains, emitted FIRST at the nm boundary: they only
            # wait on the local PV stop, so the accumulator banks free up
            # before the next n-macro's first PV matmuls need them.
            mean_sb, e2_sb = [], []
            for c2 in range(2):
                t = wrk.tile([128, NW], F32, tag="drain",
                             name=f"msb{nm}_{c2}", bufs=4)
                nc.vector.tensor_copy(t[:], mean_ps[c2][:])
                mean_sb.append(t)
            for c2 in range(2):
                t = wrk.tile([128, NW], F32, tag="drain",
                             name=f"esb{nm}_{c2}", bufs=4)
                nc.vector.tensor_copy(t[:], e2_ps[c2][:])
                e2_sb.append(t)
            return mean_sb, e2_sb

        def epilogue_red(nm, acc_d, acc_g, pt_last, drained):
            # rowsum via PE: partition-reduce both acc chains and the
            # last P tile directly -> [1, NW], broadcast back.  Emitted a
            # couple of tiles INTO the next n-macro so the in-order PE queue
            # never blocks on the acc chains at the boundary.
            red_ps = lps.tile([1, NW], F32, tag="log", name=f"red{nm}")
            nc.tensor.matmul(red_ps[:], ones_r[:, 0:1], acc_d[:],
                             start=True, stop=False)
            nc.tensor.matmul(red_ps[:], ones_r[:, 0:1], acc_g[:],
                             start=False, stop=False)
            nc.tensor.matmul(red_ps[:], ones_h[:], pt_last[:],
                             start=False, stop=True)
            rs_r = rsml.tile([1, NW], F32R, tag="rs", name=f"rsr{nm}")
            nc.vector.tensor_copy(rs_r[:], red_ps[:])
            bc_ps = lps.tile([128, NW], F32, tag="log", name=f"bc{nm}")
            nc.tensor.matmul(bc_ps[:], ones_r[0:1, :], rs_r[:],
                             start=True, stop=True)
            rcp = wrk.tile([128, NW], F32, tag="rcp", name=f"rcp{nm}", bufs=4)
            nc.vector.reciprocal_approx_fast(rcp[:], bc_ps[:])
            if drained is None:
                return rcp, [mean_ps[0], mean_ps[1]], [e2_ps[0], e2_ps[1]]
            return (rcp,) + drained

        def epilogue_tail(nm, ep):
            # last n-macro: nothing overlaps this, so pipeline the chain in
            # 128-col quarters across DVE/GpSimd/ACT/DMA to shorten the tail.
            rcp, mean_sb, e2_sb = ep
            QW = NW // 4
            var = wrk.tile([128, 2, NW], F32, tag="var", name=f"vrt", bufs=4)
            std = wrk.tile([128, 2, NW], F32, tag="var", name=f"stt", bufs=4)
            mns = [wrk.tile([128, NW], F32, tag="wrk", name=f"mnt_{c2}")
                   for c2 in range(2)]
            sqs = [wrk.tile([128, NW], F32, tag="wrk", name=f"sqt_{c2}")
                   for c2 in range(2)]
            ots = [outp.tile([128, NW], F32, tag="out", name=f"ott_{c2}")
                   for c2 in range(2)]
            HB = NW // 2
            for h in range(2):
                hs = slice(h * HB, (h + 1) * HB)
                for c2 in range(2):
                    # PSUM reads must stay on DVE (GpSimd has no PSUM port)
                    nc.vector.tensor_mul(mns[c2][:, hs], mean_sb[c2][:, hs],
                                         rcp[:, hs])
                    nc.vector.tensor_mul(var[:, c2, hs], e2_sb[c2][:, hs],
                                         rcp[:, hs])
                for c2, eng in ((0, nc.vector), (1, nc.gpsimd)):
                    eng.tensor_mul(sqs[c2][:, hs], mns[c2][:, hs],
                                   mns[c2][:, hs])
                    eng.tensor_sub(var[:, c2, hs], var[:, c2, hs],
                                   sqs[c2][:, hs])
                    # DVE tensor_tensor max: TENSOR_SCALAR MAX measures ~8x
                    # slower here, and TT max is not a legal GpSimd opcode
                    nc.vector.tensor_tensor(var[:, c2, hs], var[:, c2, hs],
                                            zeros_f[:, hs],
                                            op=mybir.AluOpType.max)
                nc.scalar.sqrt(std[:, :, hs], var[:, :, hs])
                for c2, eng in ((0, nc.vector), (1, nc.gpsimd)):
                    nsl2 = slice(nm * NW + h * HB, nm * NW + (h + 1) * HB)
                    eng.tensor_mul(ots[c2][:, hs], std[:, c2, hs],
                                   nrm_s[:, c2, nsl2])
                    eng.tensor_add(ots[c2][:, hs], ots[c2][:, hs],
                                   mns[c2][:, hs])
                    # final DMAs are the kernel tail: quarter-split across
                    # BOTH issue queues so 4 engines move each half-tile.
                    q0 = slice(h * HB, h * HB + HB // 2)
                    q1 = slice(h * HB + HB // 2, (h + 1) * HB)
                    nc.gpsimd.dma_start(out[nm, c2][:, q0], ots[c2][:, q0])
                    nc.sync.dma_start(out[nm, c2][:, q1], ots[c2][:, q1])

        def epilogue_b1(nm, ep):
            rcp, mean_sb, e2_sb = ep
            mns = []
            var = wrk.tile([128, 2, NW], F32, tag="var", name=f"vr{nm}", bufs=4)
            for c2 in range(2):
                mn = wrk.tile([128, NW], F32, tag="wrk", name=f"mn{nm}_{c2}")
                nc.vector.tensor_mul(mn[:], mean_sb[c2][:], rcp[:])
                mns.append(mn)
                e2t = var[:, c2, :]
                nc.vector.tensor_mul(e2t, e2_sb[c2][:], rcp[:])
                sq = wrk.tile([128, NW], F32, tag="wrk", name=f"sq{nm}_{c2}")
                nc.vector.tensor_mul(sq[:], mn[:], mn[:])
                nc.vector.tensor_sub(e2t, e2t, sq[:])
                nc.vector.tensor_scalar_max(e2t, e2t, 0.0)
            return var, mns

        def epilogue_b2(nm, bstate):
            # ONE fused sqrt for both c-halves, emitted long after its var
            # input is ready: the in-order ACT queue never waits on it, so
            # the sqrt-table round-trip rides pipeline slack.
            var, mns = bstate
            std = wrk.tile([128, 2, NW], F32, tag="var", name=f"st{nm}", bufs=4)
            nc.scalar.sqrt(std[:], var[:])
            return std, mns

        def epilogue_b3(nm, bstate):
            std, mns = bstate
            nsl = slice(nm * NW, (nm + 1) * NW)
            for c2 in range(2):
                ot = outp.tile([128, NW], F32, tag="out", name=f"ot{nm}_{c2}")
                nc.vector.tensor_mul(ot[:], std[:, c2, :], nrm_s[:, c2, nsl])
                nc.vector.tensor_add(ot[:], ot[:], mns[c2][:])
                nc.gpsimd.dma_start(out[nm, c2][:, 0:256], ot[:, 0:256])
                nc.gpsimd.dma_start(out[nm, c2][:, 256:512], ot[:, 256:512])

        # -------- content streaming/stats (ride the j-loop) --------
        def ct_own_dma(q):
            csl = slice(q * NW, (q + 1) * NW)
            nc.gpsimd.dma_start(ct_sh[:, :, csl], ct[q])

        def ct_own_stats(q):
            # deferred into quiet nm=1 slots: ct_sh persists, and running
            # bn_stats in the j-loop clogs DVE ahead of the conv drains the
            # logits matmuls wait on (measured 3.5us PE stalls + HAM dips)
            csl = slice(q * NW, (q + 1) * NW)
            for k in range(2):
                nc.vector.bn_stats(bnbuf[:, k, q], ct_sh[:, k, csl])

        def ct_other_chunk(o):
            tcn = ctchk.tile([128, 2, NW], F32, tag="ctchk", name=f"ctx{o}")
            nc.gpsimd.dma_start(tcn[:], ct[NM + o])
            stats_chunk(tcn, 4 + o)

        def stats_finalize_dve():
            mv = sml.tile([128, 2, 2], F32, tag="sml2", name="mv2")
            for k in range(2):
                nc.vector.bn_aggr(mv[:, k], bnbuf[:, k])
            var = sml.tile([128, 2], F32, tag="sml2", name="var2")
            nc.vector.tensor_scalar(var[:], mv[:, :, 1], HW / (HW - 1.0), EPS,
                                    op0=mybir.AluOpType.mult,
                                    op1=mybir.AluOpType.add)
            return var, mv

        def stats_finalize_act(fin):
            var, mv = fin
            sd = sml.tile([128, 2], F32, tag="sml2", name="sd2")
            nc.scalar.sqrt(sd[:], var[:])            # one fused sqrt
            rstd = sml.tile([128, 2], F32, tag="sml2", name="rstd2")
            nc.vector.reciprocal(rstd[:], sd[:])
            nmr = sml.tile([128, 2], F32, tag="sml2", name="nmr2")
            nc.vector.scalar_tensor_tensor(nmr[:], mv[:, :, 0], -1.0, rstd[:],
                                           op0=mybir.AluOpType.mult,
                                           op1=mybir.AluOpType.mult)
            return nmr, rstd

        def nrm_piece(k, nmsl, nrms):
            nmr, rstd = nrms
            csl = slice(nmsl * NW, (nmsl + 1) * NW)
            nc.vector.tensor_scalar(nrm_s[:, k, csl], ct_sh[:, k, csl],
                                    rstd[:, k:k + 1], nmr[:, k:k + 1],
                                    op0=mybir.AluOpType.mult,
                                    op1=mybir.AluOpType.add)

        # ---------------- fused staging + nm=0 ----------------
        acc0_d = accp.tile([128, NW], F32R, tag="acc", name="pacc0d")
        acc0_g = accp.tile([128, NW], F32R, tag="acc", name="pacc0g")
        fq_conv(0, t0)

        pt_last = None
        def stage(j):
            tg = chk.tile([128, 2, NW], F16, tag="chk", name=f"sk{j}")
            nc.sync.dma_start(tg[:], sk[j])
            g_conv(j, tg)
            tv = chk.tile([128, 2, NW], F16, tag="chk", name=f"st{j}")
            nc.gpsimd.dma_start(tv[:], st[j])
            v_conv(j, tv)
            if 1 <= j <= 3:
                tq = chk.tile([128, 2, NW], F16, tag="chk", name=f"ck{j}")
                nc.sync.dma_start(tq[:], ck[j])
                fq_conv(j, tq)

        # chunk staging is emitted ONE chunk ahead of the attention tiles
        # consuming the previous chunk: the scheduler runs ready work in
        # priority order, so this keeps the convs ahead of the attention
        # wavefront instead of trickling into its PE stalls.
        stage(0)
        for j in range(NCH):
            if j + 1 < NCH:
                stage(j + 1)
            for mt in range(j * NM, (j + 1) * NM):
                pt_last = attn_mt(0, mt, acc0_d, acc0_g)
                # content streaming + stats ride the exp gaps, ~one chunk
                # per j: other half at j=0..3, own half at j=3..6.
                if mt % 4 == 1 and j <= 3:
                    ct_other_chunk(j)
                if mt % 4 == 3 and 3 <= j <= 6:
                    ct_own_dma(j - 3)

        # ---------------- nm = 1..3, epilogues interleaved ----------------
        # Software-pipelined across ALL remaining tiles, including nm
        # boundaries: logits/exp are emitted one tile ahead of their PV
        # consumer, and the epilogue hooks are deferred deep enough into the
        # next n-macro that the out-of-order Tile scheduler cannot park the
        # ACT sqrt (plus its two table reloads) in the boundary idle slot
        # where it would head-of-line-block the next n-macro's exps.
        pend = (0, acc0_d, acc0_g, pt_last, epilogue_drain(0))
        fin = None
        prev = None
        nrms = None
        bstate = None
        ESKEW = 3
        LA = 3          # logits/exp lookahead depth (needs LA+1 lps banks)
        seq = [(nm, mt) for nm in range(1, NM) for mt in range(MT)]
        ahead = [attn_log(*seq[i]) for i in range(LA)]
        acc_d = acc_g = None
        e2q = []
        for idx, (nm, mt) in enumerate(seq):
            if mt == 0:
                acc_d = accp.tile([128, NW], F32R, tag="acc",
                                  name=f"pacc{nm}d")
                acc_g = accp.tile([128, NW], F32R, tag="acc",
                                  name=f"pacc{nm}g")
            pt_cur = ahead.pop(0)
            if idx + LA < len(seq):
                ahead.append(attn_log(*seq[idx + LA]))
            attn_pv_mean(mt, pt_cur)
            e2q.append((mt, pt_cur))
            if mt >= ESKEW:
                attn_pv_e2(*e2q.pop(0))
            attn_acc(mt, pt_cur, acc_d, acc_g)
            if mt == 8:
                prev = (pend[0], epilogue_red(*pend))
                if nm >= 2:
                    nrm_piece(0, nm - 1, nrms)
                    nrm_piece(1, nm - 1, nrms)
            if nm == NM - 1 and mt == 9:
                nrm_piece(0, NM - 1, nrms)
                nrm_piece(1, NM - 1, nrms)
            if mt == 11:
                bstate = epilogue_b1(*prev)
            if nm == 1 and mt in (10, 12, 14, 16):
                ct_own_stats((mt - 10) // 2)
            if mt == 17:
                bstate = epilogue_b2(prev[0], bstate)
            if nm == 1 and mt == 18:
                fin = stats_finalize_dve()
            if nm == 1 and mt == 19:
                nrms = stats_finalize_act(fin)
            if nm == 1 and mt == 20:
                nrm_piece(0, 0, nrms)
                nrm_piece(1, 0, nrms)
            if mt == (22 if nm == 1 else 19):
                epilogue_b3(prev[0], bstate)
            if mt == MT - 1:
                for item in e2q:
                    attn_pv_e2(*item)
                e2q = []
                if nm < NM - 1:
                    pend = (nm, acc_d, acc_g, pt_cur, epilogue_drain(nm))
                else:
                    prev = (nm, epilogue_red(nm, acc_d, acc_g, pt_cur, None))
        epilogue_tail(*prev)


def kernel(content, style, content_key, style_key, Wf, bf, Wg, bg, Wh, bh):
    content = np.ascontiguousarray(np.asarray(content, dtype=np.float32))
    style = np.ascontiguousarray(np.asarray(style, dtype=np.float32))
    content_key = np.ascontiguousarray(np.asarray(content_key, dtype=np.float32))
    style_key = np.ascontiguousarray(np.asarray(style_key, dtype=np.float32))
    Wf = np.asarray(Wf, dtype=np.float32)
    Wg = np.asarray(Wg, dtype=np.float32)
    Wh = np.asarray(Wh, dtype=np.float32)
    bf = np.asarray(bf, dtype=np.float32)
    bg = np.asarray(bg, dtype=np.float32)
    bh = np.asarray(bh, dtype=np.float32)

    def wlay(W):  # [O, C] -> [128, 2, 256] with [p, k, c_out] = W[c_out, k*128+p]
        return np.ascontiguousarray(W.T.reshape(2, 128, C).transpose(1, 0, 2))

    def blay(b):  # [256] -> [128, 2]
        return np.ascontiguousarray(b.reshape(2, 128).T)

    def chunk_lay(x, nch):  # [256, nch*512] -> [nch, 128, 2(k), 512]
        return np.ascontiguousarray(
            x.reshape(2, 128, nch, NW).transpose(2, 1, 0, 3))

    wall_l = np.ascontiguousarray(
        np.stack([wlay(Wf), wlay(Wg), wlay(Wh)], axis=1)).astype(np.float16)
    ball_l = np.ascontiguousarray(np.concatenate(
        [blay(bf), blay(bg), np.broadcast_to(bh, (128, C))], axis=1))

    in_maps = []
    for core in range(8):
        b, half = core // 2, core % 2
        off = half * NSH
        ctb = content[b].reshape(C, HW)
        ct_rot = np.concatenate([ctb[:, off:], ctb[:, :off]], axis=1) if off else ctb
        ck_sh = content_key[b].reshape(C, HW)[:, off:off + NSH]
        in_maps.append({
            "ck": chunk_lay(ck_sh, NM).astype(np.float16),
            "ct": chunk_lay(ct_rot, NCH),
            "sk": chunk_lay(style_key[b].reshape(C, HW), NCH).astype(np.float16),
            "st": chunk_lay(style[b].reshape(C, HW), NCH).astype(np.float16),
            "wall": wall_l, "ball": ball_l,
        })

    nc = _build_nc()
    trace = bool(os.environ.get("KERNEL_TRACE"))
    res = run_bass_kernel_spmd(nc, in_maps, core_ids=list(range(8)), trace=trace)
    _last_result.clear()
    _last_result["exec_time_ns"] = res.exec_time_ns
    _last_result["trace"] = res.instructions_and_trace

    outp = np.empty((B, C, HW), dtype=np.float32)
    for core in range(8):
        b, half = core // 2, core % 2
        o = res.results[core]["out"]          # [nm, c2, 128, 512]
        full = o.transpose(1, 2, 0, 3).reshape(C, NSH)
        outp[b, :, half * NSH:(half + 1) * NSH] = full
    return outp.reshape(B, C, 64, 64)

